# revision 1
# baseline (speedup 1.0000x reference)
"""Batched dynamic-filter cross-correlation on 8 Trainium2 NeuronCores.

Each sample b of x[128, 384, 384, 1] is VALID-correlated with its own
8x8 filter k[b] -> out[128, 377, 377, 1].

Strategy (pure data parallel, batch sharded 16 samples/core):
  out[i, j] = sum_{p,q} x[i+p, j+q] * k[p, q]
is computed per sample as 8 PSUM-accumulating TensorE matmuls (one per
filter column q) over output row-blocks of M=121 (K=M+7=128 input rows
on the contraction/partition dim):
  psum[m, n] += Band_q[k, m]^T . x[ibase+k, q+n]
with Band_q[k, m] = kern[k-m, q] for 0 <= k-m < 8, a banded Toeplitz
matrix built on host. The width shift q is a free AP offset on the rhs.
The 14 leftover output rows of 4 samples are packed into one
block-diagonal matmul group (K=4*21, M=4*14) so they cost 8 matmuls per
4 samples instead of 32. float32r (TF32-style multiplies, fp32 layout,
fp32 accumulate) runs 1 cycle/row at even N>=256; N=378 is computed and
377 columns stored.
"""

import numpy as np

B, H, W = 128, 384, 384
KH, KW = 8, 8
HO, WO = H - KH + 1, W - KW + 1          # 377, 377
N_CORES = 8
SPC = B // N_CORES                        # 16 samples per core

MAIN_BLOCKS = [(0, 121, 128), (121, 121, 128), (242, 121, 128)]
TB, TM, TK = 363, 14, 21                  # tail rows: out 363..376, in 363..383
GS = 4                                    # tail-group size (samples per group)
NO2 = WO + 1                              # 378: fp32r needs even moving dim
XW = 386                                  # x tile width (q=7 reads col 384)

_cache = {}


def _build_program():
    import concourse.mybir as mybir
    import concourse.tile as tile
    from concourse import bacc

    f32r = mybir.dt.float32r
    f32 = mybir.dt.float32
    nc = bacc.Bacc(None, target_bir_lowering=False)
    x_d = nc.dram_tensor("x", [SPC, H, W], f32r, kind="ExternalInput")
    b_d = nc.dram_tensor("bands", [SPC, 128, KW, 121], f32r, kind="ExternalInput")
    t_d = nc.dram_tensor(
        "tailbands", [SPC // GS, GS * TK, KW, GS * TM], f32r, kind="ExternalInput"
    )
    o_d = nc.dram_tensor("out", [SPC, HO, WO], f32, kind="ExternalOutput")

    with tile.TileContext(nc) as tc:
        with (
            tc.tile_pool(name="xp", bufs=8) as xp,
            tc.tile_pool(name="bp", bufs=3) as bp,
            tc.tile_pool(name="tbp", bufs=2) as tbp,
            tc.tile_pool(name="pp", bufs=8, space="PSUM") as pp,
            tc.tile_pool(name="op", bufs=6) as op,
        ):
            for g in range(SPC // GS):
                for j in range(GS):
                    s = g * GS + j
                    bt = bp.tile([128, KW, 121], f32r)
                    nc.scalar.dma_start(out=bt[:], in_=b_d[s])
                    for obase, M, K in MAIN_BLOCKS:
                        xt = xp.tile([128, XW], f32r)
                        nc.vector.memset(xt[:, W:XW].bitcast(f32), 0.0)
                        nc.gpsimd.dma_start(
                            out=xt[:K, :W], in_=x_d[s, obase : obase + K, :]
                        )
                        ps = pp.tile([128, NO2], f32)
                        for q in range(KW):
                            nc.tensor.matmul(
                                ps[:M, :NO2],
                                bt[:K, q, :M],
                                xt[:K, q : q + NO2],
                                start=(q == 0),
                                stop=(q == KW - 1),
                            )
                        ot = op.tile([128, WO], f32)
                        nc.vector.tensor_copy(out=ot[:M, :], in_=ps[:M, :WO])
                        nc.sync.dma_start(
                            out=o_d[s, obase : obase + M, :], in_=ot[:M, :]
                        )
                # tail group: GS samples' last 14 rows, block-diagonal matmul
                tt = tbp.tile([GS * TK, KW, GS * TM], f32r)
                nc.scalar.dma_start(out=tt[:], in_=t_d[g])
                xtt = xp.tile([128, XW], f32r)
                nc.vector.memset(xtt[:, W:XW].bitcast(f32), 0.0)
                for j in range(GS):
                    nc.gpsimd.dma_start(
                        out=xtt[TK * j : TK * j + TK, :W],
                        in_=x_d[g * GS + j, TB : TB + TK, :],
                    )
                ps = pp.tile([128, NO2], f32)
                for q in range(KW):
                    nc.tensor.matmul(
                        ps[: GS * TM, :NO2],
                        tt[: GS * TK, q, : GS * TM],
                        xtt[: GS * TK, q : q + NO2],
                        start=(q == 0),
                        stop=(q == KW - 1),
                    )
                ot = op.tile([128, WO], f32)
                nc.vector.tensor_copy(out=ot[: GS * TM, :], in_=ps[: GS * TM, :WO])
                for j in range(GS):
                    nc.sync.dma_start(
                        out=o_d[g * GS + j, TB : TB + TM, :],
                        in_=ot[TM * j : TM * j + TM, :],
                    )

    nc.compile()
    return nc


def _build_runner():
    """Build nc + a persistent jitted PJRT callable (compiles once)."""
    import jax
    from jax.sharding import Mesh, PartitionSpec
    from jax.experimental.shard_map import shard_map
    import concourse.mybir as mybir
    from concourse import bass2jax

    nc = _build_program()
    bass2jax.install_neuronx_cc_hook()

    partition_name = nc.partition_id_tensor.name if nc.partition_id_tensor else None

    in_names, out_names, out_avals, zero_shapes = [], [], [], []
    for alloc in nc.m.functions[0].allocations:
        if not isinstance(alloc, mybir.MemoryLocationSet):
            continue
        name = alloc.memorylocations[0].name
        if alloc.kind == "ExternalInput":
            if name != partition_name:
                in_names.append(name)
        elif alloc.kind == "ExternalOutput":
            shape = tuple(alloc.tensor_shape)
            dtype = mybir.dt.np(alloc.dtype)
            out_names.append(name)
            out_avals.append(jax.core.ShapedArray(shape, dtype))
            zero_shapes.append((shape, dtype))
    n_params = len(in_names)
    n_outs = len(out_avals)
    all_in_names = list(in_names) + list(out_names)
    if partition_name is not None:
        all_in_names.append(partition_name)
    donate = tuple(range(n_params, n_params + n_outs))

    def _body(*args):
        operands = list(args)
        if partition_name is not None:
            operands.append(bass2jax.partition_id_tensor())
        outs = bass2jax._bass_exec_p.bind(
            *operands,
            out_avals=tuple(out_avals),
            in_names=tuple(all_in_names),
            out_names=tuple(out_names),
            lowering_input_output_aliases=(),
            sim_require_finite=True,
            sim_require_nnan=True,
            nc=nc,
        )
        return tuple(outs)

    devices = jax.devices()[:N_CORES]
    mesh = Mesh(np.asarray(devices), ("core",))
    in_specs = (PartitionSpec("core"),) * (n_params + n_outs)
    out_specs = (PartitionSpec("core"),) * n_outs
    sharded = jax.jit(
        shard_map(
            _body, mesh=mesh, in_specs=in_specs, out_specs=out_specs, check_rep=False
        ),
        keep_unused=True,
    )
    del donate  # outputs are fully written by the NEFF; no donation needed

    from jax.sharding import NamedSharding

    zero_sharding = NamedSharding(mesh, PartitionSpec("core"))
    dev_zeros = [
        jax.device_put(np.zeros((N_CORES * s[0], *s[1:]), d), zero_sharding)
        for (s, d) in zero_shapes
    ]

    def run(in_maps):
        concat_in = [
            np.concatenate([np.asarray(m[name]) for m in in_maps], axis=0)
            for name in in_names
        ]
        out_arrs = sharded(*concat_in, *dev_zeros)
        return [
            {
                name: np.asarray(out_arrs[i]).reshape(
                    N_CORES, *out_avals[i].shape
                )[c]
                for i, name in enumerate(out_names)
            }
            for c in range(N_CORES)
        ]

    return nc, run


def _build_bands(k2):
    """k2: [B, 8, 8] -> bands [B, 128, 8, 121], bands[b, m+p, q, m] = k2[b, p, q]."""
    bands = np.zeros((k2.shape[0], 128, KW, 121), np.float32)
    m = np.arange(121)
    for p in range(KH):
        bands[:, m + p, :, m] = k2[:, p, :]
    return bands


def _build_tailbands(k2):
    """k2: [N, 8, 8] -> block-diag tail bands [N//GS, GS*21, 8, GS*14]."""
    n = k2.shape[0]
    tb = np.zeros((n // GS, GS * TK, KW, GS * TM), np.float32)
    m = np.arange(TM)
    for g in range(n // GS):
        for j in range(GS):
            for p in range(KH):
                tb[g, TK * j + m + p, :, TM * j + m] = k2[g * GS + j, p, :]
    return tb


def kernel(x, k):
    x = np.asarray(x, dtype=np.float32).reshape(B, H, W)
    k = np.asarray(k, dtype=np.float32).reshape(B, KH, KW)

    if "runner" not in _cache:
        _cache["runner"] = _build_runner()
    _nc, run = _cache["runner"]

    bands = _build_bands(k)
    tailbands = _build_tailbands(k)
    n_groups = SPC // GS
    in_maps = [
        {
            "x": np.ascontiguousarray(x[c * SPC : (c + 1) * SPC]),
            "bands": bands[c * SPC : (c + 1) * SPC],
            "tailbands": tailbands[c * n_groups : (c + 1) * n_groups],
        }
        for c in range(N_CORES)
    ]
    results = run(in_maps)
    out = np.concatenate([r["out"] for r in results], axis=0)
    return out.reshape(B, HO, WO, 1)



# revision 3
# speedup vs baseline: 1.0269x; 1.0269x over previous
"""Batched dynamic-filter cross-correlation on 8 Trainium2 NeuronCores.

Each sample b of x[128, 384, 384, 1] is VALID-correlated with its own
8x8 filter k[b] -> out[128, 377, 377, 1].

Strategy (pure data parallel, batch sharded 16 samples/core):
  out[i, j] = sum_{p,q} x[i+p, j+q] * k[p, q]
is computed per sample as 8 PSUM-accumulating TensorE matmuls (one per
filter column q) over output row-blocks of M=121 (K=M+7=128 input rows
on the contraction/partition dim):
  psum[m, n] += Band_q[k, m]^T . x[ibase+k, q+n]
with Band_q[k, m] = kern[k-m, q] for 0 <= k-m < 8, a banded Toeplitz
matrix built on host. The width shift q is a free AP offset on the rhs.
The 14 leftover output rows of 4 samples are packed into one
block-diagonal matmul group (K=4*21, M=4*14) so they cost 8 matmuls per
4 samples instead of 32. All operands are stored as bf16 (x, bands and
the output tensor), halving HBM traffic vs fp32; accumulation stays
fp32 in PSUM, host upcasts the result.
"""

import numpy as np
import ml_dtypes

BF16 = ml_dtypes.bfloat16

B, H, W = 128, 384, 384
KH, KW = 8, 8
HO, WO = H - KH + 1, W - KW + 1          # 377, 377
N_CORES = 8
SPC = B // N_CORES                        # 16 samples per core

MAIN_BLOCKS = [(0, 121, 128), (121, 121, 128), (242, 121, 128)]
TB, TM, TK = 363, 14, 21                  # tail rows: out 363..376, in 363..383
GS = 4                                    # tail-group size (samples per group)
NO2 = WO + 1                              # 378: even moving dim
XW = 386                                  # x tile width (q=7 reads col 384)

_cache = {}


def _build_program():
    import concourse.mybir as mybir
    import concourse.tile as tile
    from concourse import bacc

    bf16 = mybir.dt.bfloat16
    f32 = mybir.dt.float32
    nc = bacc.Bacc(None, target_bir_lowering=False)
    x_d = nc.dram_tensor("x", [SPC, H, W], bf16, kind="ExternalInput")
    b_d = nc.dram_tensor("bands", [SPC, 128, KW, 121], bf16, kind="ExternalInput")
    t_d = nc.dram_tensor(
        "tailbands", [SPC // GS, GS * TK, KW, GS * TM], bf16, kind="ExternalInput"
    )
    o_d = nc.dram_tensor("out", [SPC, HO, WO], bf16, kind="ExternalOutput")

    with tile.TileContext(nc) as tc:
        with (
            tc.tile_pool(name="xp", bufs=8) as xp,
            tc.tile_pool(name="bp", bufs=3) as bp,
            tc.tile_pool(name="tbp", bufs=2) as tbp,
            tc.tile_pool(name="pp", bufs=8, space="PSUM") as pp,
            tc.tile_pool(name="op", bufs=6) as op,
        ):
            ncopy = 0
            for g in range(SPC // GS):
                for j in range(GS):
                    s = g * GS + j
                    bt = bp.tile([128, KW, 121], bf16)
                    nc.scalar.dma_start(out=bt[:], in_=b_d[s])
                    for obase, M, K in MAIN_BLOCKS:
                        xt = xp.tile([128, XW], bf16)
                        nc.vector.memset(xt[:, W:XW], 0.0)
                        nc.gpsimd.dma_start(
                            out=xt[:K, :W], in_=x_d[s, obase : obase + K, :]
                        )
                        ps = pp.tile([128, NO2], f32)
                        for q in range(KW):
                            nc.tensor.matmul(
                                ps[:M, :NO2],
                                bt[:K, q, :M],
                                xt[:K, q : q + NO2],
                                start=(q == 0),
                                stop=(q == KW - 1),
                            )
                        ot = op.tile([128, WO], bf16)
                        if ncopy % 2 == 0:
                            nc.vector.tensor_copy(out=ot[:M, :], in_=ps[:M, :WO])
                        else:
                            nc.scalar.copy(out=ot[:M, :], in_=ps[:M, :WO])
                        ncopy += 1
                        nc.sync.dma_start(
                            out=o_d[s, obase : obase + M, :], in_=ot[:M, :]
                        )
                # tail group: GS samples' last 14 rows, block-diagonal matmul
                tt = tbp.tile([GS * TK, KW, GS * TM], bf16)
                nc.scalar.dma_start(out=tt[:], in_=t_d[g])
                xtt = xp.tile([128, XW], bf16)
                nc.vector.memset(xtt[:, W:XW], 0.0)
                for j in range(GS):
                    nc.gpsimd.dma_start(
                        out=xtt[TK * j : TK * j + TK, :W],
                        in_=x_d[g * GS + j, TB : TB + TK, :],
                    )
                ps = pp.tile([128, NO2], f32)
                for q in range(KW):
                    nc.tensor.matmul(
                        ps[: GS * TM, :NO2],
                        tt[: GS * TK, q, : GS * TM],
                        xtt[: GS * TK, q : q + NO2],
                        start=(q == 0),
                        stop=(q == KW - 1),
                    )
                ot = op.tile([128, WO], bf16)
                if ncopy % 2 == 0:
                    nc.vector.tensor_copy(out=ot[: GS * TM, :], in_=ps[: GS * TM, :WO])
                else:
                    nc.scalar.copy(out=ot[: GS * TM, :], in_=ps[: GS * TM, :WO])
                ncopy += 1
                for j in range(GS):
                    nc.sync.dma_start(
                        out=o_d[g * GS + j, TB : TB + TM, :],
                        in_=ot[TM * j : TM * j + TM, :],
                    )

    nc.compile()
    return nc


def _build_runner():
    """Build nc + a persistent jitted PJRT callable (compiles once)."""
    import jax
    from jax.sharding import Mesh, PartitionSpec
    from jax.experimental.shard_map import shard_map
    import concourse.mybir as mybir
    from concourse import bass2jax

    nc = _build_program()
    bass2jax.install_neuronx_cc_hook()

    partition_name = nc.partition_id_tensor.name if nc.partition_id_tensor else None

    in_names, out_names, out_avals, zero_shapes = [], [], [], []
    for alloc in nc.m.functions[0].allocations:
        if not isinstance(alloc, mybir.MemoryLocationSet):
            continue
        name = alloc.memorylocations[0].name
        if alloc.kind == "ExternalInput":
            if name != partition_name:
                in_names.append(name)
        elif alloc.kind == "ExternalOutput":
            shape = tuple(alloc.tensor_shape)
            dtype = mybir.dt.np(alloc.dtype)
            out_names.append(name)
            out_avals.append(jax.core.ShapedArray(shape, dtype))
            zero_shapes.append((shape, dtype))
    n_params = len(in_names)
    n_outs = len(out_avals)
    all_in_names = list(in_names) + list(out_names)
    if partition_name is not None:
        all_in_names.append(partition_name)

    def _body(*args):
        operands = list(args)
        if partition_name is not None:
            operands.append(bass2jax.partition_id_tensor())
        outs = bass2jax._bass_exec_p.bind(
            *operands,
            out_avals=tuple(out_avals),
            in_names=tuple(all_in_names),
            out_names=tuple(out_names),
            lowering_input_output_aliases=(),
            sim_require_finite=True,
            sim_require_nnan=True,
            nc=nc,
        )
        return tuple(outs)

    devices = jax.devices()[:N_CORES]
    mesh = Mesh(np.asarray(devices), ("core",))
    in_specs = (PartitionSpec("core"),) * (n_params + n_outs)
    out_specs = (PartitionSpec("core"),) * n_outs
    sharded = jax.jit(
        shard_map(
            _body, mesh=mesh, in_specs=in_specs, out_specs=out_specs, check_rep=False
        ),
        keep_unused=True,
    )

    from jax.sharding import NamedSharding

    zero_sharding = NamedSharding(mesh, PartitionSpec("core"))
    dev_zeros = [
        jax.device_put(np.zeros((N_CORES * s[0], *s[1:]), d), zero_sharding)
        for (s, d) in zero_shapes
    ]

    def run(in_maps):
        concat_in = [
            np.concatenate([np.asarray(m[name]) for m in in_maps], axis=0)
            for name in in_names
        ]
        out_arrs = sharded(*concat_in, *dev_zeros)
        return [
            {
                name: np.asarray(out_arrs[i]).reshape(
                    N_CORES, *out_avals[i].shape
                )[c]
                for i, name in enumerate(out_names)
            }
            for c in range(N_CORES)
        ]

    return nc, run


def _build_bands(k2):
    """k2: [B, 8, 8] -> bands [B, 128, 8, 121], bands[b, m+p, q, m] = k2[b, p, q]."""
    bands = np.zeros((k2.shape[0], 128, KW, 121), BF16)
    m = np.arange(121)
    k2 = k2.astype(BF16)
    for p in range(KH):
        bands[:, m + p, :, m] = k2[:, p, :]
    return bands


def _build_tailbands(k2):
    """k2: [N, 8, 8] -> block-diag tail bands [N//GS, GS*21, 8, GS*14]."""
    n = k2.shape[0]
    tb = np.zeros((n // GS, GS * TK, KW, GS * TM), BF16)
    m = np.arange(TM)
    k2 = k2.astype(BF16)
    for g in range(n // GS):
        for j in range(GS):
            for p in range(KH):
                tb[g, TK * j + m + p, :, TM * j + m] = k2[g * GS + j, p, :]
    return tb


def kernel(x, k):
    x = np.asarray(x, dtype=np.float32).reshape(B, H, W)
    k = np.asarray(k, dtype=np.float32).reshape(B, KH, KW)

    if "runner" not in _cache:
        _cache["runner"] = _build_runner()
    _nc, run = _cache["runner"]

    xb = x.astype(BF16)
    bands = _build_bands(k)
    tailbands = _build_tailbands(k)
    n_groups = SPC // GS
    in_maps = [
        {
            "x": np.ascontiguousarray(xb[c * SPC : (c + 1) * SPC]),
            "bands": bands[c * SPC : (c + 1) * SPC],
            "tailbands": tailbands[c * n_groups : (c + 1) * n_groups],
        }
        for c in range(N_CORES)
    ]
    results = run(in_maps)
    out = np.concatenate([r["out"] for r in results], axis=0)
    return out.astype(np.float32).reshape(B, HO, WO, 1)
